# revision 55
# baseline (speedup 1.0000x reference)
"""Sharded KNN retrieval (NeighborhoodAggregation) on 8 TRN2 NeuronCores.

Reference computation:
    x   = normalize(features)            # [B, D]
    dis = x @ feat_memory.T              # [B, N]
    dis[b, idx[b]] = global_min          # self-mask
    top5 = top_k(dis, 5).indices
    mean_logits  = mean(pred_memory[top5], axis=1)
    pseudo_labels = argmax(mean_logits, 1)

Device strategy (FAISS-style sharded search, per sharding hint):
  - feat_memory is sharded row-wise across the 8 cores (12500 rows each).
  - Row normalization of `features` is skipped: dis rows are scaled by a
    positive per-row constant, which leaves per-row rankings unchanged, and
    only rankings feed the output.
  - Each core computes dis_local = features @ shard.T with bf16 inputs and
    fp32 PSUM accumulation, and reduces each 2500-wide stripe to its top-8
    (values + indices) with the DVE max/max_index instructions.
  - Host merges the 8*40 candidates per row, drops the self-index (reference
    sets it to the global min, which can never reach top-5 of 100k), rescores
    the best 40 in fp32 for rank robustness, gathers pred_memory and reduces.
"""

import sys

for _p in (
    "/root/.axon_site",
    "/root/.axon_site/_ro/trn_rl_repo",
    "/root/.axon_site/_ro/pypackages",
    "/opt/trn_rl_repo",
    "/opt/pypackages",
):
    if _p not in sys.path:
        sys.path.append(_p)

import os

import numpy as np
import ml_dtypes

LDW_OPT = os.environ.get("KNN_LDW_OPT", "0") == "1"
LDW_SKIP = os.environ.get("KNN_LDW_SKIP", "0") == "1"
# Software-interleaved DoubleRow: stationary weights pre-interleaved on the
# host so the PE weight load is a contiguous read.
SWI = os.environ.get("KNN_SWI", "1") == "1"

# concourse's trace path does `from antenv.axon_hooks import ...`; some
# images lack that module entirely. Provide a None-hook shim so tracing
# degrades gracefully instead of raising.
try:
    import antenv.axon_hooks  # noqa: F401
except ImportError:
    import types

    try:
        import antenv

        _hooks = types.ModuleType("antenv.axon_hooks")
        _hooks._hook = None
        _hooks.set_axon_ntff_profile_hook = lambda h: setattr(_hooks, "_hook", h)
        _hooks.get_axon_ntff_profile_hook = lambda: _hooks._hook
        sys.modules["antenv.axon_hooks"] = _hooks
        antenv.axon_hooks = _hooks
    except ImportError:
        pass

import concourse.bacc as bacc
import concourse.mybir as mybir
from concourse.tile import TileContext
import concourse.bass_utils as _bass_utils
from concourse.bass_utils import run_bass_kernel_spmd

if LDW_OPT and not getattr(_bass_utils, "_knn_ldw_patch", False):
    # let walrus elide LDWEIGHTS for consecutive matmuls that share the
    # same stationary operand (our k-group-outer loop repeats it 5x)
    _orig_run_command = _bass_utils.run_command

    def _run_command_ldw(argv, **kwargs):
        if isinstance(argv, list):
            argv = [
                "--enable-ldw-opt=true" if a == "--enable-ldw-opt=false" else a
                for a in argv
            ]
        return _orig_run_command(argv, **kwargs)

    _bass_utils.run_command = _run_command_ldw
    _bass_utils._knn_ldw_patch = True

# Problem sizes (hardcoded per contest contract)
B = 1024
D = 1024
N = 100000
C_CORES = 8
NL = N // C_CORES  # 12500 rows of feat_memory per core
K = 5

P = 128
CHUNK = 500  # matmul moving free dim (one PSUM bank holds 512 fp32)
STAGE_CHUNKS = 5  # max chunks per top-8 stripe
# Variable stripe widths: narrow first stripes -> the opening matmuls only
# wait on small DMAs (stage-1 load must fit under stage-0's compute);
# narrow last stripe -> short tail MAX8.
STAGE_WIDTHS = [500, 1500, 2500, 2500, 2500, 2000, 1000]
assert sum(STAGE_WIDTHS) == NL
N_STAGES = len(STAGE_WIDTHS)
STAGE_BASES = [sum(STAGE_WIDTHS[:i]) for i in range(N_STAGES)]
CAND = 8 * N_STAGES  # candidates per row per core

BF16 = mybir.dt.bfloat16
F32 = mybir.dt.float32
U32 = mybir.dt.uint32
FP8 = mybir.dt.float8e4

IN_DT = FP8  # matmul input dtype (fp8 e4m3 + DoubleRow)
STAGE_DT = BF16  # dis staging dtype for the DVE top-8 scan
W_SCALE = 32.0  # keeps fp8-cast bank rows out of the subnormal range


def _pad16(n):
    return (n + 15) // 16 * 16


def interleave_xT(xT, k_groups, b_tiles):
    """Host layout for DoubleRowSwInterleave stationaries.

    xT is [d, b]. For each k-group g and b-tile, the PE expects the two
    k-rows' weight columns interleaved pairwise and column-reversed:
    element (2j + i) of the 256-wide block = lhsT[p, i, 127 - j].
    Returns [k_groups, 128, b_tiles * 256].
    """
    d, b = xT.shape
    a = xT.reshape(k_groups, 2, 128, b_tiles, 128)  # [g, i, p, bt, m]
    rev = a[..., ::-1]                               # m -> 127 - j
    out = np.transpose(rev, (0, 2, 3, 4, 1))         # [g, p, bt, j, i]
    return np.ascontiguousarray(out.reshape(k_groups, 128, b_tiles * 256))


def build_nc(b=B, d=D, nl=NL, chunk=CHUNK, stage_chunks=STAGE_CHUNKS,
             in_dt=IN_DT, stage_dt=STAGE_DT, stage_widths=None):
    """Per-core Bass module: dis = x @ w_shard.T, striped top-8.

    fp8 inputs run the PE in DoubleRow mode: each matmul contracts a pair
    of 128-row k-tiles ([128, 2, M] / [128, 2, N] APs, middle step % 16 == 0).
    """
    b_tiles = b // P
    k_tiles = d // P
    if stage_widths is None:
        stage_widths = [chunk * stage_chunks] * (nl // (chunk * stage_chunks))
    assert sum(stage_widths) == nl and all(w % chunk == 0 for w in stage_widths)
    n_stages = len(stage_widths)
    stage_max = max(stage_widths)
    max_chunks = stage_max // chunk
    assert max_chunks <= stage_chunks
    cand_w = 8 * n_stages
    double_row = in_dt == FP8
    kg = 2 if double_row else 1  # k-tiles per matmul group
    k_groups = k_tiles // kg

    swi = double_row and SWI

    nc = bacc.Bacc("TRN2", target_bir_lowering=False, debug=False)
    if swi:
        xT_d = nc.dram_tensor(
            "xT", [k_groups, P, b_tiles * 2 * P], in_dt, kind="ExternalInput"
        )
    else:
        xT_d = nc.dram_tensor("xT", [d, b], in_dt, kind="ExternalInput")
    wt_d = nc.dram_tensor("wt", [d, nl], in_dt, kind="ExternalInput")
    # each candidate is one f32 word: [bf16(exp(dis)) bits | 16-bit local idx]
    val_d = nc.dram_tensor("cand_val", [b, cand_w], F32, kind="ExternalOutput")

    # [d, n] viewed as [p, ko, n] so the contraction dim lands on partitions
    if not swi:
        xT_v = xT_d.ap().rearrange("(ko p) b -> p ko b", p=P)
    wt_v = wt_d.ap().rearrange("(ko p) n -> p ko n", p=P)

    if swi:
        perf_mode = mybir.MatmulPerfMode.DoubleRowSwInterleave
    elif double_row:
        perf_mode = mybir.MatmulPerfMode.DoubleRow
    else:
        perf_mode = None
    N_COMB = 3

    with TileContext(nc) as tc:
        with (
            tc.tile_pool(name="const", bufs=1) as const_pool,
            tc.tile_pool(name="wt", bufs=2) as wt_pool,
            tc.tile_pool(name="cand", bufs=1) as cand_pool,
            tc.tile_pool(name="psum", bufs=1, space="PSUM") as psum_pool,
        ):
            # Combined-word stripes: lane0 (low u16) = iota prefilled once,
            # lane1 (high u16) = bf16 bits of exp(dis) written per stripe.
            # exp > 0 makes every word a positive f32, so one MAX8 returns
            # the top-8 (score, index) pairs — no FIND_INDEX8 pass needed.
            comb = []
            for i in range(N_COMB):
                ct = cand_pool.tile(
                    [P, stage_max], U32,
                    name=f"comb{'L' if LDW_OPT else ''}{i}",
                )
                lanes = ct.bitcast(mybir.dt.uint16).rearrange(
                    "p (n two) -> p n two", two=2
                )
                if i == 0:
                    nc.gpsimd.iota(
                        lanes[:, :, 0], pattern=[[1, stage_max]], base=0,
                        channel_multiplier=0,
                    )
                    lanes0 = lanes
                else:
                    # clone comb0's iota lane with a DVE copy — a second
                    # 4.3us GpSimd iota would stall the first MAX8s
                    nc.vector.tensor_copy(lanes[:, :, 0], lanes0[:, :, 0])
                comb.append(ct)

            def load_wt_stage(s):
                w = stage_widths[s]
                base = sum(stage_widths[:s])
                tiles = []
                for g in range(k_groups):
                    wg = wt_pool.tile(
                        [P, kg, _pad16(stage_max)], in_dt, name="wt_sb",
                        tag=f"wt{g}",
                    )
                    nc.sync.dma_start(
                        wg[:, :, :w],
                        wt_v[:, g * kg : (g + 1) * kg, base : base + w],
                    )
                    tiles.append(wg)
                return tiles

            # stage-0 weights first so the opening matmul isn't stuck
            # behind the xT transfers in the DMA queues
            wt_stage0 = load_wt_stage(0)

            # Warm-up matmuls on zeroed tiles while the input DMAs land:
            # keeps the PE HAM window busy so the real stream starts at
            # the full 2.4 GHz clock instead of the cold 1.2 GHz.
            if double_row:
                dummy_w = const_pool.tile([P, 2 * P], in_dt, name="dummy_w")
                dummy_r = const_pool.tile([P, kg, chunk], in_dt, name="dummy_r")
                nc.vector.memset(dummy_w[:], 0)
                nc.vector.memset(dummy_r[:], 0)
                wu_ps = psum_pool.tile([P, chunk], F32, name="wu_ps",
                                       tag="ps0", bufs=2)
                for _ in range(8):
                    nc.tensor.matmul(
                        wu_ps[:],
                        lhsT=dummy_w[:] if swi else
                        dummy_w[:].rearrange("p (two m) -> p two m", two=2),
                        rhs=dummy_r[:],
                        start=True,
                        stop=True,
                        perf_mode=perf_mode,
                    )

            xT_sb = []
            for g in range(k_groups):
                if swi:
                    xg = const_pool.tile(
                        [P, b_tiles * 2 * P], in_dt, name=f"xT_sb{g}"
                    )
                    nc.sync.dma_start(xg[:], xT_d.ap()[g])
                else:
                    xg = const_pool.tile([P, kg, b], in_dt, name=f"xT_sb{g}")
                    nc.sync.dma_start(xg[:], xT_v[:, g * kg : (g + 1) * kg, :])
                xT_sb.append(xg)

            cval = [
                cand_pool.tile([P, cand_w], F32, name=f"cval{bt}")
                for bt in range(b_tiles)
            ]

            unit = 0
            n_units = n_stages * b_tiles
            for s in range(n_stages):
                w = stage_widths[s]
                n_chunks = w // chunk
                wt_sb = wt_stage0 if s == 0 else load_wt_stage(s)
                for bt in range(b_tiles):
                    # k-group outer, chunk inner: the stationary operand
                    # repeats across chunks. The very last unit goes
                    # chunk-major instead so its ACT work overlaps its
                    # matmuls and the kernel tail stays short.
                    pss = [
                        psum_pool.tile(
                            [P, chunk], F32, name="ps", tag=f"ps{c}",
                            bufs=2 if c < 8 - stage_chunks else 1,
                        )
                        for c in range(n_chunks)
                    ]
                    ct = comb[unit % N_COMB]
                    ct_bf = ct.bitcast(mybir.dt.bfloat16).rearrange(
                        "p (n two) -> p n two", two=2
                    )

                    def mm(g, c):
                        if swi:
                            lhsT = xT_sb[g][:, bt * 2 * P : (bt + 1) * 2 * P]
                            rhs = wt_sb[g][:, :, c * chunk : (c + 1) * chunk]
                        elif double_row:
                            lhsT = xT_sb[g][:, :, bt * P : (bt + 1) * P]
                            rhs = wt_sb[g][:, :, c * chunk : (c + 1) * chunk]
                        else:
                            lhsT = xT_sb[g][:, 0, bt * P : (bt + 1) * P]
                            rhs = wt_sb[g][:, 0, c * chunk : (c + 1) * chunk]
                        nc.tensor.matmul(
                            pss[c][:],
                            lhsT=lhsT,
                            rhs=rhs,
                            start=(g == 0),
                            stop=(g == k_groups - 1),
                            perf_mode=perf_mode,
                        )

                    def act(c):
                        nc.scalar.activation(
                            ct_bf[:, c * chunk : (c + 1) * chunk, 1],
                            pss[c][:],
                            mybir.ActivationFunctionType.Exp,
                            scale=1.0 / W_SCALE,
                        )

                    if unit == n_units - 1:
                        for c in range(n_chunks):
                            for g in range(k_groups):
                                mm(g, c)
                            act(c)
                    else:
                        for g in range(k_groups):
                            for c in range(n_chunks):
                                mm(g, c)
                        for c in range(n_chunks):
                            act(c)
                    nc.vector.max(
                        out=cval[bt][:, s * 8 : (s + 1) * 8],
                        in_=ct.bitcast(F32)[:, :w],
                    )
                    unit += 1

            for bt in range(b_tiles):
                nc.sync.dma_start(val_d.ap()[bt * P : (bt + 1) * P, :], cval[bt][:])

    nc.compile()
    return nc


_NC_CACHE = {}


def _get_nc():
    if "nc" not in _NC_CACHE:
        _NC_CACHE["nc"] = build_nc(stage_widths=STAGE_WIDTHS)
    return _NC_CACHE["nc"]


def _device_candidates(features, feat_memory, **run_kwargs):
    """Run the sharded search; returns (values [B, 8*CAND], global idx [B, 8*CAND])."""
    np_in = mybir.dt.np(IN_DT)
    xT = np.ascontiguousarray(features.T).astype(np_in)
    if SWI and IN_DT == FP8:
        xT = interleave_xT(xT, (D // P) // 2, B // P)
    in_maps = []
    for c in range(C_CORES):
        shard = feat_memory[c * NL : (c + 1) * NL]
        # global positive scale: rank-invariant, avoids fp8 subnormals
        wt = (np.ascontiguousarray(shard.T) * W_SCALE).astype(np_in)
        in_maps.append({"xT": xT, "wt": wt})

    nc = _get_nc()
    res = run_bass_kernel_spmd(nc, in_maps, core_ids=list(range(C_CORES)), **run_kwargs)

    # decode combined words: low u16 = local index, high u16 = bf16 exp-score
    words = np.concatenate(
        [
            np.ascontiguousarray(res.results[c]["cand_val"]).view(np.uint32)
            for c in range(C_CORES)
        ],
        axis=1,
    )
    lidx = (words & 0xFFFF).astype(np.int64)
    vals = (
        (words >> 16).astype(np.uint16).view(ml_dtypes.bfloat16).astype(np.float32)
    )
    # local stripe index -> global row index
    base = np.concatenate(
        [
            np.repeat(np.array(STAGE_BASES) + c * NL, 8)
            for c in range(C_CORES)
        ]
    )
    gidx = lidx + base[None, :]
    _NC_CACHE["last_results"] = res
    return vals, gidx, res


def kernel(features, idx, feat_memory, pred_memory):
    features = np.asarray(features, dtype=np.float32)
    feat_memory = np.asarray(feat_memory, dtype=np.float32)
    pred_memory = np.asarray(pred_memory, dtype=np.float32)
    idx = np.asarray(idx).astype(np.int64)

    vals, gidx, _ = _device_candidates(features, feat_memory)

    # Drop self-index candidates (reference masks them to the global min,
    # which cannot appear in the top-5 of 100k entries).
    vals = np.where(gidx == idx[:, None], -np.inf, vals)

    # Keep the 40 best per row by device (bf16) score, then rescore those
    # exactly in fp32 so close ranks are decided at full precision.
    R = 40
    part = np.argpartition(-vals, R - 1, axis=1)[:, :R]
    cand_i = np.take_along_axis(gidx, part, axis=1)  # [B, R]
    cand_v = np.take_along_axis(vals, part, axis=1)
    cand_vecs = feat_memory[cand_i]  # [B, R, D]
    exact = np.einsum("brd,bd->br", cand_vecs, features, dtype=np.float32)
    exact = np.where(np.isinf(cand_v), -np.inf, exact)

    # top-5, ties broken by smaller global index (jax.lax.top_k convention)
    order = np.lexsort((cand_i, -exact), axis=1)[:, :K]
    top5 = np.take_along_axis(cand_i, order, axis=1)  # [B, K]

    _NC_CACHE["last_top5"] = top5
    mean_logits = pred_memory[top5].mean(axis=1, dtype=np.float32)
    pseudo_labels = np.argmax(mean_logits, axis=1).astype(np.int32)
    return pseudo_labels, mean_logits.astype(np.float32)


# revision 57
# speedup vs baseline: 1.0151x; 1.0151x over previous
"""Sharded KNN retrieval (NeighborhoodAggregation) on 8 TRN2 NeuronCores.

Reference computation:
    x   = normalize(features)            # [B, D]
    dis = x @ feat_memory.T              # [B, N]
    dis[b, idx[b]] = global_min          # self-mask
    top5 = top_k(dis, 5).indices
    mean_logits  = mean(pred_memory[top5], axis=1)
    pseudo_labels = argmax(mean_logits, 1)

Device strategy (FAISS-style sharded search, per sharding hint):
  - feat_memory is sharded row-wise across the 8 cores (12500 rows each).
  - Row normalization of `features` is skipped: dis rows are scaled by a
    positive per-row constant, which leaves per-row rankings unchanged, and
    only rankings feed the output.
  - Each core computes dis_local = features @ shard.T with bf16 inputs and
    fp32 PSUM accumulation, and reduces each 2500-wide stripe to its top-8
    (values + indices) with the DVE max/max_index instructions.
  - Host merges the 8*40 candidates per row, drops the self-index (reference
    sets it to the global min, which can never reach top-5 of 100k), rescores
    the best 40 in fp32 for rank robustness, gathers pred_memory and reduces.
"""

import sys

for _p in (
    "/root/.axon_site",
    "/root/.axon_site/_ro/trn_rl_repo",
    "/root/.axon_site/_ro/pypackages",
    "/opt/trn_rl_repo",
    "/opt/pypackages",
):
    if _p not in sys.path:
        sys.path.append(_p)

import os

import numpy as np
import ml_dtypes

LDW_OPT = os.environ.get("KNN_LDW_OPT", "0") == "1"
LDW_SKIP = os.environ.get("KNN_LDW_SKIP", "0") == "1"
# Software-interleaved DoubleRow: stationary weights pre-interleaved on the
# host so the PE weight load is a contiguous read.
SWI = os.environ.get("KNN_SWI", "1") == "1"

# concourse's trace path does `from antenv.axon_hooks import ...`; some
# images lack that module entirely. Provide a None-hook shim so tracing
# degrades gracefully instead of raising.
try:
    import antenv.axon_hooks  # noqa: F401
except ImportError:
    import types

    try:
        import antenv

        _hooks = types.ModuleType("antenv.axon_hooks")
        _hooks._hook = None
        _hooks.set_axon_ntff_profile_hook = lambda h: setattr(_hooks, "_hook", h)
        _hooks.get_axon_ntff_profile_hook = lambda: _hooks._hook
        sys.modules["antenv.axon_hooks"] = _hooks
        antenv.axon_hooks = _hooks
    except ImportError:
        pass

import concourse.bacc as bacc
import concourse.mybir as mybir
from concourse.tile import TileContext
import concourse.bass_utils as _bass_utils
from concourse.bass_utils import run_bass_kernel_spmd

if LDW_OPT and not getattr(_bass_utils, "_knn_ldw_patch", False):
    # let walrus elide LDWEIGHTS for consecutive matmuls that share the
    # same stationary operand (our k-group-outer loop repeats it 5x)
    _orig_run_command = _bass_utils.run_command

    def _run_command_ldw(argv, **kwargs):
        if isinstance(argv, list):
            argv = [
                "--enable-ldw-opt=true" if a == "--enable-ldw-opt=false" else a
                for a in argv
            ]
        return _orig_run_command(argv, **kwargs)

    _bass_utils.run_command = _run_command_ldw
    _bass_utils._knn_ldw_patch = True

# Problem sizes (hardcoded per contest contract)
B = 1024
D = 1024
N = 100000
C_CORES = 8
NL = N // C_CORES  # 12500 rows of feat_memory per core
K = 5

P = 128
CHUNK = 500  # matmul moving free dim (one PSUM bank holds 512 fp32)
STAGE_CHUNKS = 5  # max chunks per top-8 stripe
# Variable stripe widths: narrow first stripes -> the opening matmuls only
# wait on small DMAs (stage-1 load must fit under stage-0's compute);
# narrow last stripe -> short tail MAX8.
STAGE_WIDTHS = [500, 1500, 2500, 2500, 2500, 2000, 1000]
assert sum(STAGE_WIDTHS) == NL
N_STAGES = len(STAGE_WIDTHS)
STAGE_BASES = [sum(STAGE_WIDTHS[:i]) for i in range(N_STAGES)]
CAND = 8 * N_STAGES  # candidates per row per core

BF16 = mybir.dt.bfloat16
F32 = mybir.dt.float32
U32 = mybir.dt.uint32
FP8 = mybir.dt.float8e4

IN_DT = FP8  # matmul input dtype (fp8 e4m3 + DoubleRow)
STAGE_DT = BF16  # dis staging dtype for the DVE top-8 scan
W_SCALE = 32.0  # keeps fp8-cast bank rows out of the subnormal range


def _pad16(n):
    return (n + 15) // 16 * 16


def interleave_xT(xT, k_groups, b_tiles):
    """Host layout for DoubleRowSwInterleave stationaries.

    xT is [d, b]. For each k-group g and b-tile, the PE expects the two
    k-rows' weight columns interleaved pairwise and column-reversed:
    element (2j + i) of the 256-wide block = lhsT[p, i, 127 - j].
    Returns [k_groups, 128, b_tiles * 256].
    """
    d, b = xT.shape
    a = xT.reshape(k_groups, 2, 128, b_tiles, 128)  # [g, i, p, bt, m]
    rev = a[..., ::-1]                               # m -> 127 - j
    out = np.transpose(rev, (0, 2, 3, 4, 1))         # [g, p, bt, j, i]
    return np.ascontiguousarray(out.reshape(k_groups, 128, b_tiles * 256))


def build_nc(b=B, d=D, nl=NL, chunk=CHUNK, stage_chunks=STAGE_CHUNKS,
             in_dt=IN_DT, stage_dt=STAGE_DT, stage_widths=None):
    """Per-core Bass module: dis = x @ w_shard.T, striped top-8.

    fp8 inputs run the PE in DoubleRow mode: each matmul contracts a pair
    of 128-row k-tiles ([128, 2, M] / [128, 2, N] APs, middle step % 16 == 0).
    """
    b_tiles = b // P
    k_tiles = d // P
    if stage_widths is None:
        stage_widths = [chunk * stage_chunks] * (nl // (chunk * stage_chunks))
    assert sum(stage_widths) == nl and all(w % chunk == 0 for w in stage_widths)
    n_stages = len(stage_widths)
    stage_max = max(stage_widths)
    max_chunks = stage_max // chunk
    assert max_chunks <= stage_chunks
    cand_w = 8 * n_stages
    double_row = in_dt == FP8
    kg = 2 if double_row else 1  # k-tiles per matmul group
    k_groups = k_tiles // kg

    swi = double_row and SWI

    nc = bacc.Bacc("TRN2", target_bir_lowering=False, debug=False)
    if swi:
        xT_d = nc.dram_tensor(
            "xT", [k_groups, P, b_tiles * 2 * P], in_dt, kind="ExternalInput"
        )
    else:
        xT_d = nc.dram_tensor("xT", [d, b], in_dt, kind="ExternalInput")
    wt_d = nc.dram_tensor("wt", [d, nl], in_dt, kind="ExternalInput")
    # each candidate is one f32 word: [bf16(exp(dis)) bits | 16-bit local idx]
    val_d = nc.dram_tensor("cand_val", [b, cand_w], F32, kind="ExternalOutput")

    # [d, n] viewed as [p, ko, n] so the contraction dim lands on partitions
    if not swi:
        xT_v = xT_d.ap().rearrange("(ko p) b -> p ko b", p=P)
    wt_v = wt_d.ap().rearrange("(ko p) n -> p ko n", p=P)

    if swi:
        perf_mode = mybir.MatmulPerfMode.DoubleRowSwInterleave
    elif double_row:
        perf_mode = mybir.MatmulPerfMode.DoubleRow
    else:
        perf_mode = None
    N_COMB = 3

    with TileContext(nc) as tc:
        with (
            tc.tile_pool(name="const", bufs=1) as const_pool,
            tc.tile_pool(name="wt", bufs=2) as wt_pool,
            tc.tile_pool(name="cand", bufs=1) as cand_pool,
            tc.tile_pool(name="psum", bufs=1, space="PSUM") as psum_pool,
        ):
            # Combined-word stripes: lane0 (low u16) = iota prefilled once,
            # lane1 (high u16) = bf16 bits of exp(dis) written per stripe.
            # exp > 0 makes every word a positive f32, so one MAX8 returns
            # the top-8 (score, index) pairs — no FIND_INDEX8 pass needed.
            comb = []
            for i in range(N_COMB):
                ct = cand_pool.tile(
                    [P, stage_max], U32,
                    name=f"comb{'L' if LDW_OPT else ''}{i}",
                )
                lanes = ct.bitcast(mybir.dt.uint16).rearrange(
                    "p (n two) -> p n two", two=2
                )
                if i == 0:
                    nc.gpsimd.iota(
                        lanes[:, :, 0], pattern=[[1, stage_max]], base=0,
                        channel_multiplier=0,
                    )
                    lanes0 = lanes
                else:
                    # clone comb0's iota lane with a DVE copy — a second
                    # 4.3us GpSimd iota would stall the first MAX8s
                    nc.vector.tensor_copy(lanes[:, :, 0], lanes0[:, :, 0])
                comb.append(ct)

            def load_wt_stage(s):
                w = stage_widths[s]
                base = sum(stage_widths[:s])
                tiles = []
                for g in range(k_groups):
                    wg = wt_pool.tile(
                        [P, kg, _pad16(stage_max)], in_dt, name="wt_sb",
                        tag=f"wt{g}",
                    )
                    nc.sync.dma_start(
                        wg[:, :, :w],
                        wt_v[:, g * kg : (g + 1) * kg, base : base + w],
                    )
                    tiles.append(wg)
                return tiles

            # stage-0 weights first so the opening matmul isn't stuck
            # behind the xT transfers in the DMA queues
            wt_stage0 = load_wt_stage(0)

            # Warm-up matmuls on zeroed tiles while the input DMAs land:
            # keeps the PE HAM window busy so the real stream starts at
            # the full 2.4 GHz clock instead of the cold 1.2 GHz.
            if double_row:
                dummy_w = const_pool.tile([P, 2 * P], in_dt, name="dummy_w")
                dummy_r = const_pool.tile([P, kg, chunk], in_dt, name="dummy_r")
                nc.vector.memset(dummy_w[:], 0)
                nc.vector.memset(dummy_r[:], 0)
                wu_ps = psum_pool.tile([P, chunk], F32, name="wu_ps",
                                       tag="ps0", bufs=2)
                for _ in range(int(os.environ.get("KNN_WU", "16"))):
                    nc.tensor.matmul(
                        wu_ps[:],
                        lhsT=dummy_w[:] if swi else
                        dummy_w[:].rearrange("p (two m) -> p two m", two=2),
                        rhs=dummy_r[:],
                        start=True,
                        stop=True,
                        perf_mode=perf_mode,
                    )

            xT_sb = []
            for g in range(k_groups):
                if swi:
                    xg = const_pool.tile(
                        [P, b_tiles * 2 * P], in_dt, name=f"xT_sb{g}"
                    )
                    nc.sync.dma_start(xg[:], xT_d.ap()[g])
                else:
                    xg = const_pool.tile([P, kg, b], in_dt, name=f"xT_sb{g}")
                    nc.sync.dma_start(xg[:], xT_v[:, g * kg : (g + 1) * kg, :])
                xT_sb.append(xg)

            cval = [
                cand_pool.tile([P, cand_w], F32, name=f"cval{bt}")
                for bt in range(b_tiles)
            ]

            unit = 0
            n_units = n_stages * b_tiles
            for s in range(n_stages):
                w = stage_widths[s]
                n_chunks = w // chunk
                wt_sb = wt_stage0 if s == 0 else load_wt_stage(s)
                for bt in range(b_tiles):
                    # k-group outer, chunk inner: the stationary operand
                    # repeats across chunks. The very last unit goes
                    # chunk-major instead so its ACT work overlaps its
                    # matmuls and the kernel tail stays short.
                    pss = [
                        psum_pool.tile(
                            [P, chunk], F32, name="ps", tag=f"ps{c}",
                            bufs=2 if c < 8 - stage_chunks else 1,
                        )
                        for c in range(n_chunks)
                    ]
                    ct = comb[unit % N_COMB]
                    ct_bf = ct.bitcast(mybir.dt.bfloat16).rearrange(
                        "p (n two) -> p n two", two=2
                    )

                    def mm(g, c):
                        if swi:
                            lhsT = xT_sb[g][:, bt * 2 * P : (bt + 1) * 2 * P]
                            rhs = wt_sb[g][:, :, c * chunk : (c + 1) * chunk]
                        elif double_row:
                            lhsT = xT_sb[g][:, :, bt * P : (bt + 1) * P]
                            rhs = wt_sb[g][:, :, c * chunk : (c + 1) * chunk]
                        else:
                            lhsT = xT_sb[g][:, 0, bt * P : (bt + 1) * P]
                            rhs = wt_sb[g][:, 0, c * chunk : (c + 1) * chunk]
                        nc.tensor.matmul(
                            pss[c][:],
                            lhsT=lhsT,
                            rhs=rhs,
                            start=(g == 0),
                            stop=(g == k_groups - 1),
                            perf_mode=perf_mode,
                        )

                    def act(c):
                        nc.scalar.activation(
                            ct_bf[:, c * chunk : (c + 1) * chunk, 1],
                            pss[c][:],
                            mybir.ActivationFunctionType.Exp,
                            scale=1.0 / W_SCALE,
                        )

                    if unit == n_units - 1:
                        for c in range(n_chunks):
                            for g in range(k_groups):
                                mm(g, c)
                            act(c)
                    else:
                        for g in range(k_groups):
                            for c in range(n_chunks):
                                mm(g, c)
                        for c in range(n_chunks):
                            act(c)
                    nc.vector.max(
                        out=cval[bt][:, s * 8 : (s + 1) * 8],
                        in_=ct.bitcast(F32)[:, :w],
                    )
                    unit += 1

            for bt in range(b_tiles):
                nc.sync.dma_start(val_d.ap()[bt * P : (bt + 1) * P, :], cval[bt][:])

    nc.compile()
    return nc


_NC_CACHE = {}


def _get_nc():
    if "nc" not in _NC_CACHE:
        _NC_CACHE["nc"] = build_nc(stage_widths=STAGE_WIDTHS)
    return _NC_CACHE["nc"]


def _device_candidates(features, feat_memory, **run_kwargs):
    """Run the sharded search; returns (values [B, 8*CAND], global idx [B, 8*CAND])."""
    np_in = mybir.dt.np(IN_DT)
    xT = np.ascontiguousarray(features.T).astype(np_in)
    if SWI and IN_DT == FP8:
        xT = interleave_xT(xT, (D // P) // 2, B // P)
    in_maps = []
    for c in range(C_CORES):
        shard = feat_memory[c * NL : (c + 1) * NL]
        # global positive scale: rank-invariant, avoids fp8 subnormals
        wt = (np.ascontiguousarray(shard.T) * W_SCALE).astype(np_in)
        in_maps.append({"xT": xT, "wt": wt})

    nc = _get_nc()
    res = run_bass_kernel_spmd(nc, in_maps, core_ids=list(range(C_CORES)), **run_kwargs)

    # decode combined words: low u16 = local index, high u16 = bf16 exp-score
    words = np.concatenate(
        [
            np.ascontiguousarray(res.results[c]["cand_val"]).view(np.uint32)
            for c in range(C_CORES)
        ],
        axis=1,
    )
    lidx = (words & 0xFFFF).astype(np.int64)
    vals = (
        (words >> 16).astype(np.uint16).view(ml_dtypes.bfloat16).astype(np.float32)
    )
    # local stripe index -> global row index
    base = np.concatenate(
        [
            np.repeat(np.array(STAGE_BASES) + c * NL, 8)
            for c in range(C_CORES)
        ]
    )
    gidx = lidx + base[None, :]
    _NC_CACHE["last_results"] = res
    return vals, gidx, res


def kernel(features, idx, feat_memory, pred_memory):
    features = np.asarray(features, dtype=np.float32)
    feat_memory = np.asarray(feat_memory, dtype=np.float32)
    pred_memory = np.asarray(pred_memory, dtype=np.float32)
    idx = np.asarray(idx).astype(np.int64)

    vals, gidx, _ = _device_candidates(features, feat_memory)

    # Drop self-index candidates (reference masks them to the global min,
    # which cannot appear in the top-5 of 100k entries).
    vals = np.where(gidx == idx[:, None], -np.inf, vals)

    # Keep the 40 best per row by device (bf16) score, then rescore those
    # exactly in fp32 so close ranks are decided at full precision.
    R = 40
    part = np.argpartition(-vals, R - 1, axis=1)[:, :R]
    cand_i = np.take_along_axis(gidx, part, axis=1)  # [B, R]
    cand_v = np.take_along_axis(vals, part, axis=1)
    cand_vecs = feat_memory[cand_i]  # [B, R, D]
    exact = np.einsum("brd,bd->br", cand_vecs, features, dtype=np.float32)
    exact = np.where(np.isinf(cand_v), -np.inf, exact)

    # top-5, ties broken by smaller global index (jax.lax.top_k convention)
    order = np.lexsort((cand_i, -exact), axis=1)[:, :K]
    top5 = np.take_along_axis(cand_i, order, axis=1)  # [B, K]

    _NC_CACHE["last_top5"] = top5
    mean_logits = pred_memory[top5].mean(axis=1, dtype=np.float32)
    pseudo_labels = np.argmax(mean_logits, axis=1).astype(np.int32)
    return pseudo_labels, mean_logits.astype(np.float32)
